# revision 32
# baseline (speedup 1.0000x reference)
"""Trainium2 Bass kernel for LorentzMultiheadAttention (B=2, N=2048, H=8, D=64, E=512).

Sharding: 8 cores = 2 batches x 4 head-pairs. Core c handles batch b=c//4 and
heads {2*(c%4), 2*(c%4)+1}. Queries run through the attention pipeline in 4
quarters of 512; each quarter's per-head centroid + head-sum is
ReduceScattered over the 4-core batch group while later quarters compute,
hiding the collectives.

Key design points:
- PV and the projections use fp8e4 DoubleRow matmuls (2 contraction rows per
  cycle): PV pairs adjacent key tiles, projections pair E=512/128 subtiles.
  Scores (contract=64) gain nothing from DoubleRow, so they stay bf16.
- Weights are scaled by 32 so fp8e4 stays in normal range. The scale cancels:
  the Lorentz centroid is scale-invariant, the lift becomes
  t = sqrt(32^2 + ||x'||^2), and the score scale folds into the exp activation
  scale. The Lorentz sign is folded into negated K weights (host-side).
- Softmax exp writes fp8 directly (ACT output cast) in key-tile pairs that
  feed PV's DoubleRow layout; softmax normalization and the mean-over-heads
  divide are skipped (centroid scale-invariance).
- The epilogue transposes each quarter's PV output to natural [q, d] layout
  on the PE, computes 1/sqrt(|<ave,ave>|) ~= 1/(t - delta/(2t)) with DVE
  reciprocal_approx_fast (output is heavily time-dominated; delta uses the
  same bf16-rounded t^2 so the cancellation is exact), and reduces heads with
  a free-dim broadcast multiply. No ACT usage after the softmax exps: exactly
  2 activation-table loads (sqrt epoch, then exp).
- A burst of dummy matmuls at kernel start warms the PE HAM clock gate while
  inputs DMA in; a dummy exp right after the lift sqrts prefetches the exp
  table before the first score tile lands.
"""

import os
import sys

for _p in ("/opt/trn_rl_repo", "/root/.axon_site/_ro/trn_rl_repo"):
    if os.path.isdir(_p) and _p not in sys.path:
        sys.path.insert(0, _p)

import numpy as np

import concourse.bacc as bacc
import concourse.bass as bass
import concourse.mybir as mybir
import concourse.tile as tile

B = 2
N = 2048
H = 8
D = 64
E = 512
DM1 = D - 1  # 63
P = 128
N_CORES = 8
NQTR = 4
QTR = N // NQTR  # 512 queries per quarter
QB = 128  # query block per core from one quarter's ReduceScatter

W_SCALE = 32.0
SC2 = W_SCALE * W_SCALE  # 1024

F32 = mybir.dt.float32
BF16 = mybir.dt.bfloat16
FP8 = mybir.dt.float8e4
EXP = mybir.ActivationFunctionType.Exp
SQRT = mybir.ActivationFunctionType.Sqrt
ADD = mybir.AluOpType.add
MULT = mybir.AluOpType.mult
DR = mybir.MatmulPerfMode.DoubleRow

REPLICA_GROUPS = [[0, 1, 2, 3], [4, 5, 6, 7]]


def _emit(tc, nc, io, scale_val, bias_val):
    from contextlib import ExitStack

    ctx = ExitStack()
    with ctx:
        consts = ctx.enter_context(tc.tile_pool(name="consts", bufs=1))
        sb = ctx.enter_context(tc.tile_pool(name="sb", bufs=1))

        # ---- constants / weights to SBUF ----
        w_sb = {}
        for nm in ("wq", "wk", "wv"):
            w = consts.tile([P, 4, P], FP8, name=f"{nm}_sb")
            nc.sync.dma_start(w[:], io[nm].ap().rearrange("(c p) m -> p c m", p=P))
            w_sb[nm] = w
        b_sb = {}
        for nm in ("bq", "bk"):
            bt = consts.tile([P, 1], F32, name=f"{nm}_sb")
            nc.sync.dma_start(bt[:], io[nm].ap().rearrange("(p one) -> p one", one=1))
            b_sb[nm] = bt
        lift_mask = consts.tile([P, P], BF16)
        nc.sync.dma_start(lift_mask[:], io["lift_mask"].ap())
        ident = consts.tile([P, P], BF16)
        nc.sync.dma_start(ident[:], io["ident"].ap())
        sc2bias = consts.tile([P, 1], F32)
        nc.gpsimd.memset(sc2bias[:], SC2)
        ebias = consts.tile([P, 1], F32)

        # ---- warm-up matmuls: keep the PE HAM gate busy during input DMA ----
        ctxW = ExitStack()
        psW = ctxW.enter_context(tc.tile_pool(name="psW", bufs=1, space="PSUM"))
        warm = psW.tile([P, P], F32, tag="warm")
        for _ in range(14):
            nc.tensor.matmul(
                warm[:], lhsT=lift_mask[:], rhs=lift_mask[:], start=True, stop=True
            )
        ctxW.close()

        # ---- inputs (fp8, chunked DMA so projections can start early) ----
        xs = sb.tile([P, 4, N], FP8)
        xq = sb.tile([P, 4, N], FP8)
        for ec in range(4):
            nc.sync.dma_start(
                xs[:, ec : ec + 1, :],
                io["xs"].ap().rearrange("(c p) n -> p c n", p=P)[:, ec : ec + 1, :],
            )
        for ec in range(4):
            nc.sync.dma_start(
                xq[:, ec : ec + 1, :],
                io["xq"].ap().rearrange("(c p) n -> p c n", p=P)[:, ec : ec + 1, :],
            )

        # q/k: [128 (h*64+d), 2048] bf16, rows 0/64 become the lift time rows
        q_sb = sb.tile([P, N], BF16)
        k_sb = sb.tile([P, N], BF16)
        # v: [128 keys, 8 pairs, 2 (key tile in pair), 128 (h*64+d)] fp8
        v_sb = sb.tile([P, 8, 2, P], FP8)

        ctxA = ExitStack()
        psP = ctxA.enter_context(tc.tile_pool(name="psP", bufs=1, space="PSUM"))

        def project_T(x_sb, wname, bias, dst, tag):
            """Transposed projection: dst = W^T x + b (both heads)."""
            ps = psP.tile([P, 4, 512], F32, tag=tag)
            for pr in range(2):
                for qc in range(4):
                    nc.tensor.matmul(
                        ps[:, qc, :],
                        lhsT=w_sb[wname][:, 2 * pr : 2 * pr + 2, :],
                        rhs=x_sb[:, 2 * pr : 2 * pr + 2, qc * 512 : (qc + 1) * 512],
                        start=(pr == 0),
                        stop=(pr == 1),
                        perf_mode=DR,
                    )
            nc.vector.tensor_tensor(
                dst[:],
                ps[:].rearrange("p c n -> p (c n)"),
                bias[:].to_broadcast((P, N)),
                ADD,
            )

        def lift_T(dst):
            """Write t = sqrt(1024 + ||x_s||^2) into rows 0/64 of dst.

            The norm matmul lands both heads' norms on partitions {0,1}
            (engines can't use strided partition APs); the sqrt fills a bf16
            staging pair that a DMA scatters to partitions {0,64}."""
            sq = sb.tile([P, N], BF16, tag="liftsq")
            nc.vector.tensor_tensor(sq[:], dst[:], dst[:], MULT)
            nrm_t = psP.tile([P, 16, P], F32, tag="vps")
            nrm = nrm_t[:].rearrange("p (c u) m -> p c (u m)", u=4)
            for qc in range(4):
                nc.tensor.matmul(
                    nrm[0:2, qc, :],
                    lhsT=lift_mask[:, 0:2],
                    rhs=sq[:, qc * 512 : (qc + 1) * 512],
                    start=True,
                    stop=True,
                )
            ttmp = sb.tile([2, N], BF16, tag="ttmp")
            nc.scalar.activation(
                ttmp[:],
                nrm_t[0:2, :, :].rearrange("h c m -> h (c m)"),
                SQRT,
                bias=sc2bias[0:2, :],
                scale=1.0,
            )
            nc.sync.dma_start(
                dst[:].rearrange("(h d) n -> h d n", h=2)[:, 0, :],
                ttmp[:],
            )

        # K and V projections first (independent psum tags keep the PE fed),
        # then the K lift, Q projection + lift. V's transpose to natural
        # layout is deferred into the phase-B psT pool.
        project_T(xs, "wk", b_sb["bk"], k_sb, "kqps")
        vps = psP.tile([P, 16, P], F32, tag="vps")
        # 4 mc slots share each 2KB psum zero-region: only the first matmul
        # of a bank may set start (it marks the whole region pending-zero).
        for pr in range(2):
            for mc in range(16):
                nc.tensor.matmul(
                    vps[:, mc, :],
                    lhsT=xs[:, 2 * pr : 2 * pr + 2, mc * P : (mc + 1) * P],
                    rhs=w_sb["wv"][:, 2 * pr : 2 * pr + 2, :],
                    start=(pr == 0 and mc % 4 == 0),
                    stop=(pr == 1 and mc % 4 == 3),
                    perf_mode=DR,
                    skip_group_check=True,
                )
        nc.vector.tensor_copy(
            out=v_sb[:].rearrange("p a b m -> p (a b m)"),
            in_=vps[:].rearrange("p a m -> p (a m)"),
        )
        lift_T(k_sb)
        project_T(xq, "wq", b_sb["bq"], q_sb, "kqps")
        lift_T(q_sb)
        ctxA.close()

        psS = ctx.enter_context(tc.tile_pool(name="psS", bufs=2, space="PSUM"))
        psPV = ctx.enter_context(tc.tile_pool(name="psPV", bufs=1, space="PSUM"))
        psT = ctx.enter_context(tc.tile_pool(name="psT", bufs=2, space="PSUM"))

        vsq = sb.tile([P, 8, 2, P], BF16, tag="vsq")
        nc.vector.tensor_tensor(vsq[:], v_sb[:], v_sb[:], MULT)
        vn = sb.tile([P, 8, 2, 2, 1], F32, tag="vn")
        nc.vector.tensor_reduce(
            vn[:, :, :, :, 0],
            vsq[:].rearrange("p a b (h d) -> p a b h d", h=2),
            axis=mybir.AxisListType.X,
            op=ADD,
        )
        nc.scalar.activation(
            v_sb[:].rearrange("p a b (h d) -> p a b h d", h=2)[:, :, :, :, 0:1],
            vn[:],
            SQRT,
            bias=sc2bias[:],
            scale=1.0,
        )
        # ebias = 0*v_sb[...] + act_bias: pins every exp behind the V lift so
        # the ACT queue runs all Sqrt calls before switching to the Exp table.
        nc.vector.tensor_scalar(
            ebias[:], v_sb[:, 0, 0, 0:1], 0.0, 2.0 / scale_val + bias_val,
            MULT, ADD,
        )

        # ---- Phase B/C: attention with deferred PV + per-half ReduceScatter ----
        pP = ctx.enter_context(tc.tile_pool(name="pP", bufs=32))
        dram = ctx.enter_context(tc.tile_pool(name="dram", bufs=1, space="DRAM"))

        act_scale = -2.0 / (scale_val * SC2)

        # dummy exp: prefetch the exp table right after the sqrt epoch
        junk = sb.tile([P, 1], F32, tag="junk")
        nc.scalar.activation(junk[:], ebias[:], EXP, scale=1.0)

        cc_outs = []
        pending_rs = []

        def emit_rs(half, hs_half):
            cc_in = dram.tile([4, 256, 64], F32, name=f"cc_in{half}")
            cc_out = dram.tile([256, 64], F32, name=f"cc_out{half}")
            nc.sync.dma_start(
                cc_in[:].rearrange("g (a p) d -> p g a d", p=P),
                hs_half[:].rearrange("p (g a) d -> p g a d", g=4),
            )
            nc.gpsimd.collective_compute(
                "ReduceScatter",
                ADD,
                replica_groups=REPLICA_GROUPS,
                ins=[cc_in[:].opt()],
                outs=[cc_out[:].opt()],
            )
            cc_outs.append(cc_out)

        for half in range(2):
            q0 = half * 1024
            # -- scores + exp: all 16 key tiles x 2 heads x 2 query chunks --
            p_ts = {}
            for pr in range(8):
                for h in range(2):
                    p_t = pP.tile([P, 2, 2, 512], FP8, tag="p", name=f"pt{half}_{pr}_{h}")
                    p_ts[(pr, h)] = p_t
                    for qcl in range(2):
                        s_ps = psS.tile([P, 2, 512], F32, tag="s")
                        for mci in range(2):
                            mc = 2 * pr + mci
                            nc.tensor.matmul(
                                s_ps[:, mci, :],
                                lhsT=k_sb[
                                    h * 64 : (h + 1) * 64, mc * P : (mc + 1) * P
                                ],
                                rhs=q_sb[
                                    h * 64 : (h + 1) * 64,
                                    q0 + qcl * 512 : q0 + (qcl + 1) * 512,
                                ],
                                start=True,
                                stop=True,
                            )
                        nc.scalar.activation(
                            p_t[:, qcl, :, :], s_ps[:], EXP,
                            scale=act_scale, bias=ebias[:],
                        )

            # previous half's ReduceScatter: emitted only now so none of this
            # half's PV bookkeeping lands behind the blocking collective on
            # the gpsimd queue
            if pending_rs:
                emit_rs(*pending_rs.pop())

            # -- PV + per-head centroid + head-sum, one 512-query chunk at a --
            # -- time (pv psum tile is drained before the next chunk reuses) --
            hs_half = sb.tile([P, 8, 64], F32, tag=f"hs_half{half}")
            for qcl in range(2):
                pv = psPV.tile([64, 2, 512], F32, tag="pv")
                for pr in range(8):
                    for h in range(2):
                        nc.tensor.matmul(
                            pv[:, h, :],
                            lhsT=v_sb[:, pr, :, h * 64 : (h + 1) * 64],
                            rhs=p_ts[(pr, h)][:, qcl, :, :],
                            start=(pr == 0),
                            stop=(pr == 7),
                            perf_mode=DR,
                            skip_group_check=True,
                        )
                o_bf = sb.tile([64, 2, 512], BF16, tag="o_bf")
                nc.vector.tensor_copy(
                    out=o_bf[:].rearrange("p a n -> p (a n)"),
                    in_=pv[:].rearrange("p a n -> p (a n)"),
                )
                ps_t = psT.tile([P, 2, 4, 64], BF16, tag="ot")
                for h in range(2):
                    for c in range(4):
                        nc.tensor.transpose(
                            ps_t[:, h, c, :],
                            o_bf[:, h, c * P : (c + 1) * P],
                            ident[0:64, 0:64],
                        )
                o_nat = sb.tile([P, 4, P], BF16, tag="o_nat")
                nc.vector.tensor_copy(
                    out=o_nat[:].rearrange("p c (h d) -> p h c d", h=2),
                    in_=ps_t[:],
                )
                sq = sb.tile([P, 4, P], BF16, tag="sq_nat")
                nc.vector.tensor_tensor(sq[:], o_nat[:], o_nat[:], MULT)
                p2 = sb.tile([P, 4, 2, 1], F32, tag="p2")
                nc.vector.tensor_reduce(
                    p2[:, :, :, 0],
                    sq[:].rearrange("p a (h d) -> p a h d", h=2),
                    axis=mybir.AxisListType.X,
                    op=ADD,
                )
                tbar = sb.tile([P, 4, 2, 1], F32, tag="tbar")
                nc.vector.tensor_copy(
                    out=tbar[:].rearrange("p a h one -> p (a h one)"),
                    in_=o_nat[:].rearrange("p a (h d) -> p a h d", h=2)[
                        :, :, :, 0:1
                    ].rearrange("p a h one -> p (a h one)"),
                )
                # delta = colsum - t^2 with the *same* bf16-rounded t^2 from
                # sq, so the dominant terms cancel exactly.
                fl = lambda ap: ap.rearrange("p a h one -> p (a h) one")
                delta = sb.tile([P, 4, 2, 1], F32, tag="delta")
                nc.vector.affine_then_add(
                    out=fl(delta[:]),
                    in0=sq[:].rearrange("p a (h d) -> p (a h) d", h=2)[:, :, 0:1],
                    in1=fl(p2[:]),
                    scale=-1.0,
                    bias=0.0,
                )
                rt = sb.tile([P, 4, 2, 1], F32, tag="rt")
                nc.vector.reciprocal_approx_fast(fl(rt[:]), fl(tbar[:]))
                z = sb.tile([P, 4, 2, 1], F32, tag="z")
                nc.vector.tensor_tensor(z[:], delta[:], rt[:], MULT)
                den = sb.tile([P, 4, 2, 1], F32, tag="den")
                nc.vector.affine_then_add(
                    out=fl(den[:]), in0=fl(z[:]), in1=fl(tbar[:]),
                    scale=-0.5, bias=0.0,
                )
                rec = sb.tile([P, 4, 2, 1], F32, tag="rec")
                nc.vector.reciprocal_approx_fast(fl(rec[:]), fl(den[:]))
                o4 = o_nat[:].rearrange("p a (h d) -> p a h d", h=2)
                m0 = sb.tile([P, 4, 64], F32, tag="m0")
                nc.vector.tensor_tensor(
                    m0[:], o4[:, :, 0, :],
                    rec[:, :, 0, :].to_broadcast((P, 4, 64)), MULT,
                )
                m1 = sb.tile([P, 4, 64], F32, tag="m1")
                nc.vector.tensor_tensor(
                    m1[:], o4[:, :, 1, :],
                    rec[:, :, 1, :].to_broadcast((P, 4, 64)), MULT,
                )
                nc.vector.tensor_tensor(
                    hs_half[:, qcl * 4 : (qcl + 1) * 4, :], m0[:], m1[:], ADD
                )

            pending_rs.append((half, hs_half))
        emit_rs(*pending_rs.pop())

        # ---- final centroid on the two local 256-query slices (natural) ----
        for half in range(2):
            fin = sb.tile([P, 2, 64], F32, tag="fin")
            nc.scalar.dma_start(
                fin[:], cc_outs[half][:].rearrange("(a p) d -> p a d", p=P)
            )
            fsq = sb.tile([P, 2, 64], BF16, tag="fsq")
            nc.vector.tensor_tensor(fsq[:], fin[:], fin[:], MULT)
            fp2 = sb.tile([P, 2, 1], F32, tag="fp2")
            nc.vector.tensor_reduce(
                fp2[:, :, 0], fsq[:], axis=mybir.AxisListType.X, op=ADD
            )
            fdel = sb.tile([P, 2, 1], F32, tag="fdel")
            nc.vector.affine_then_add(
                out=fdel[:], in0=fsq[:, :, 0:1], in1=fp2[:], scale=-1.0, bias=0.0
            )
            frt = sb.tile([P, 2, 1], F32, tag="frt")
            nc.vector.reciprocal_approx_fast(frt[:], fin[:, :, 0:1])
            fz = sb.tile([P, 2, 1], F32, tag="fz")
            nc.vector.tensor_tensor(fz[:], fdel[:], frt[:], MULT)
            fden = sb.tile([P, 2, 1], F32, tag="fden")
            nc.vector.affine_then_add(
                out=fden[:], in0=fz[:], in1=fin[:, :, 0:1], scale=-0.5, bias=0.0
            )
            frec = sb.tile([P, 2, 1], F32, tag="frec")
            nc.vector.reciprocal_approx_fast(frec[:], fden[:])
            out_sb = sb.tile([P, 2, 64], F32, tag="out_sb")
            nc.vector.tensor_tensor(
                out_sb[:], fin[:], frec[:].to_broadcast((P, 2, 64)), MULT
            )
            nc.scalar.dma_start(
                io["out"].ap()[half, :, :].rearrange("(a p) d -> p a d", p=P),
                out_sb[:],
            )


def _build(scale_val, bias_val):
    nc = bacc.Bacc(num_devices=N_CORES)
    io = {}
    io["xq"] = nc.declare_dram_parameter("xq", [E, N], FP8, isOutput=False)
    io["xs"] = nc.declare_dram_parameter("xs", [E, N], FP8, isOutput=False)
    for nm in ("wq", "wk", "wv"):
        io[nm] = nc.declare_dram_parameter(nm, [E, P], FP8, isOutput=False)
    for nm in ("bq", "bk"):
        io[nm] = nc.declare_dram_parameter(nm, [P], F32, isOutput=False)
    io["lift_mask"] = nc.declare_dram_parameter("lift_mask", [P, P], BF16, isOutput=False)
    io["ident"] = nc.declare_dram_parameter("ident", [P, P], BF16, isOutput=False)
    io["out"] = nc.declare_dram_parameter("out", [2, 256, 64], F32, isOutput=True)

    with tile.TileContext(nc) as tc:
        _emit(tc, nc, io, scale_val, bias_val)
    nc.compile()
    return nc


_BUILD_CACHE = {}


def _get_nc(scale_val, bias_val):
    key = (float(scale_val), float(bias_val))
    if key not in _BUILD_CACHE:
        _BUILD_CACHE[key] = _build(*key)
    return _BUILD_CACHE[key]


def _pad_wT(w_heads):
    """w_heads: [126, 512] spatial weights for 2 heads -> [512, 128] transposed
    with zero columns at 0 and 64 (time slots), scaled by W_SCALE."""
    out = np.zeros((E, P), dtype=np.float32)
    out[:, 1:64] = W_SCALE * w_heads[0:DM1, :].T
    out[:, 65:128] = W_SCALE * w_heads[DM1 : 2 * DM1, :].T
    return np.ascontiguousarray(out)


def _pad_b(b_heads):
    out = np.zeros((P,), dtype=np.float32)
    out[1:64] = W_SCALE * b_heads[0:DM1]
    out[65:128] = W_SCALE * b_heads[DM1 : 2 * DM1]
    return out


def make_in_maps(
    query_input, source_input, Wq_w, Wq_b, Wk_w, Wk_b, Wv_w, Wv_b, scale, bias
):
    import ml_dtypes

    F8 = ml_dtypes.float8_e4m3fn
    BF = ml_dtypes.bfloat16

    lift_mask = np.zeros((P, P), dtype=np.float32)
    lift_mask[1:64, 0] = 1.0
    lift_mask[65:128, 1] = 1.0
    ident = np.eye(P, dtype=np.float32)

    in_maps = []
    for c in range(N_CORES):
        b = c // 4
        h0 = 2 * (c % 4)
        sl = slice(h0 * DM1, (h0 + 2) * DM1)
        m = {
            "xq": np.ascontiguousarray(query_input[b].T).astype(F8),
            "xs": np.ascontiguousarray(source_input[b].T).astype(F8),
            "wq": _pad_wT(Wq_w[sl]).astype(F8),
            "wk": _pad_wT(-Wk_w[sl]).astype(F8),  # Lorentz sign folded into K
            "wv": _pad_wT(Wv_w[sl]).astype(F8),
            "bq": _pad_b(Wq_b[sl]),
            "bk": _pad_b(-Wk_b[sl]),
            "lift_mask": lift_mask.astype(BF),
            "ident": ident.astype(BF),
        }
        in_maps.append(m)
    return in_maps


def kernel(
    query_input,
    source_input,
    Wq_w,
    Wq_b,
    Wk_w,
    Wk_b,
    Wv_w,
    Wv_b,
    scale,
    bias,
    _trace=False,
):
    assert not np.any(np.asarray(Wv_b)), "nonzero V bias not supported"
    scale_val = float(np.asarray(scale).reshape(-1)[0])
    bias_val = float(np.asarray(bias).reshape(-1)[0]) if np.asarray(bias).size else 0.0

    nc = _get_nc(scale_val, bias_val)
    in_maps = make_in_maps(
        query_input, source_input, Wq_w, Wq_b, Wk_w, Wk_b, Wv_w, Wv_b, scale, bias
    )

    from concourse.bass_utils import run_bass_kernel_spmd

    res = run_bass_kernel_spmd(
        nc, in_maps, core_ids=list(range(N_CORES)), trace=_trace
    )

    out = np.zeros((B, N, D), dtype=np.float32)
    for c in range(N_CORES):
        b = c // 4
        g = c % 4
        r = res.results[c]["out"]  # [2, 256, 64]
        for half in range(2):
            q0 = half * 1024 + g * 256
            out[b, q0 : q0 + 256, :] = r[half]
    if _trace:
        kernel.last_exec_time_ns = res.exec_time_ns
        kernel.last_results = res
    return out


# revision 33
# speedup vs baseline: 1.2076x; 1.2076x over previous
"""Trainium2 Bass kernel for LorentzMultiheadAttention (B=2, N=2048, H=8, D=64, E=512).

Sharding: 8 cores = 2 batches x 4 head-pairs. Core c handles batch b=c//4 and
heads {2*(c%4), 2*(c%4)+1}. Queries run through the attention pipeline in 4
quarters of 512; each quarter's per-head centroid + head-sum is
ReduceScattered over the 4-core batch group while later quarters compute,
hiding the collectives.

Key design points:
- PV and the projections use fp8e4 DoubleRow matmuls (2 contraction rows per
  cycle): PV pairs adjacent key tiles, projections pair E=512/128 subtiles.
  Scores (contract=64) gain nothing from DoubleRow, so they stay bf16.
- Weights are scaled by 32 so fp8e4 stays in normal range. The scale cancels:
  the Lorentz centroid is scale-invariant, the lift becomes
  t = sqrt(32^2 + ||x'||^2), and the score scale folds into the exp activation
  scale. The Lorentz sign is folded into negated K weights (host-side).
- Softmax exp writes fp8 directly (ACT output cast) in key-tile pairs that
  feed PV's DoubleRow layout; softmax normalization and the mean-over-heads
  divide are skipped (centroid scale-invariance).
- The epilogue transposes each quarter's PV output to natural [q, d] layout
  on the PE, computes 1/sqrt(|<ave,ave>|) ~= 1/(t - delta/(2t)) with DVE
  reciprocal_approx_fast (output is heavily time-dominated; delta uses the
  same bf16-rounded t^2 so the cancellation is exact), and reduces heads with
  a free-dim broadcast multiply. No ACT usage after the softmax exps: exactly
  2 activation-table loads (sqrt epoch, then exp).
- A burst of dummy matmuls at kernel start warms the PE HAM clock gate while
  inputs DMA in; a dummy exp right after the lift sqrts prefetches the exp
  table before the first score tile lands.
"""

import os
import sys

for _p in ("/opt/trn_rl_repo", "/root/.axon_site/_ro/trn_rl_repo"):
    if os.path.isdir(_p) and _p not in sys.path:
        sys.path.insert(0, _p)

import numpy as np

import concourse.bacc as bacc
import concourse.bass as bass
import concourse.mybir as mybir
import concourse.tile as tile

B = 2
N = 2048
H = 8
D = 64
E = 512
DM1 = D - 1  # 63
P = 128
N_CORES = 8
NQTR = 4
QTR = N // NQTR  # 512 queries per quarter
QB = 128  # query block per core from one quarter's ReduceScatter

W_SCALE = 32.0
SC2 = W_SCALE * W_SCALE  # 1024

F32 = mybir.dt.float32
BF16 = mybir.dt.bfloat16
FP8 = mybir.dt.float8e4
EXP = mybir.ActivationFunctionType.Exp
SQRT = mybir.ActivationFunctionType.Sqrt
ADD = mybir.AluOpType.add
MULT = mybir.AluOpType.mult
DR = mybir.MatmulPerfMode.DoubleRow

REPLICA_GROUPS = [[0, 1, 2, 3], [4, 5, 6, 7]]


def _emit(tc, nc, io, scale_val, bias_val):
    from contextlib import ExitStack

    ctx = ExitStack()
    with ctx:
        consts = ctx.enter_context(tc.tile_pool(name="consts", bufs=1))
        sb = ctx.enter_context(tc.tile_pool(name="sb", bufs=1))

        # ---- constants / weights to SBUF ----
        w_sb = {}
        for nm in ("wq", "wk", "wv"):
            w = consts.tile([P, 4, P], FP8, name=f"{nm}_sb")
            nc.sync.dma_start(w[:], io[nm].ap().rearrange("(c p) m -> p c m", p=P))
            w_sb[nm] = w
        b_sb = {}
        for nm in ("bq", "bk"):
            bt = consts.tile([P, 1], F32, name=f"{nm}_sb")
            nc.sync.dma_start(bt[:], io[nm].ap().rearrange("(p one) -> p one", one=1))
            b_sb[nm] = bt
        lift_mask = consts.tile([P, P], BF16)
        nc.sync.dma_start(lift_mask[:], io["lift_mask"].ap())
        ident = consts.tile([P, P], BF16)
        nc.sync.dma_start(ident[:], io["ident"].ap())
        sc2bias = consts.tile([P, 1], F32)
        nc.gpsimd.memset(sc2bias[:], SC2)
        ebias = consts.tile([P, 1], F32)

        # ---- warm-up matmuls: keep the PE HAM gate busy during input DMA ----
        ctxW = ExitStack()
        psW = ctxW.enter_context(tc.tile_pool(name="psW", bufs=1, space="PSUM"))
        warm = psW.tile([P, P], F32, tag="warm")
        for _ in range(14):
            nc.tensor.matmul(
                warm[:], lhsT=lift_mask[:], rhs=lift_mask[:], start=True, stop=True
            )
        ctxW.close()

        # ---- inputs (fp8, chunked DMA so projections can start early) ----
        xs = sb.tile([P, 4, N], FP8)
        xq = sb.tile([P, 4, N], FP8)
        for ec in range(4):
            nc.sync.dma_start(
                xs[:, ec : ec + 1, :],
                io["xs"].ap().rearrange("(c p) n -> p c n", p=P)[:, ec : ec + 1, :],
            )
        for ec in range(4):
            nc.sync.dma_start(
                xq[:, ec : ec + 1, :],
                io["xq"].ap().rearrange("(c p) n -> p c n", p=P)[:, ec : ec + 1, :],
            )

        # q/k: [128 (h*64+d), 2048] bf16, rows 0/64 become the lift time rows
        q_sb = sb.tile([P, N], BF16)
        k_sb = sb.tile([P, N], BF16)
        # v: [128 keys, 8 pairs, 2 (key tile in pair), 128 (h*64+d)] fp8
        v_sb = sb.tile([P, 8, 2, P], FP8)

        ctxA = ExitStack()
        psP = ctxA.enter_context(tc.tile_pool(name="psP", bufs=1, space="PSUM"))

        def project_T(x_sb, wname, bias, dst, tag):
            """Transposed projection: dst = W^T x + b (both heads)."""
            ps = psP.tile([P, 4, 512], F32, tag=tag)
            for pr in range(2):
                for qc in range(4):
                    nc.tensor.matmul(
                        ps[:, qc, :],
                        lhsT=w_sb[wname][:, 2 * pr : 2 * pr + 2, :],
                        rhs=x_sb[:, 2 * pr : 2 * pr + 2, qc * 512 : (qc + 1) * 512],
                        start=(pr == 0),
                        stop=(pr == 1),
                        perf_mode=DR,
                    )
            nc.vector.tensor_tensor(
                dst[:],
                ps[:].rearrange("p c n -> p (c n)"),
                bias[:].to_broadcast((P, N)),
                ADD,
            )

        def lift_T(dst):
            """Write t = sqrt(1024 + ||x_s||^2) into rows 0/64 of dst.

            The norm matmul lands both heads' norms on partitions {0,1}
            (engines can't use strided partition APs); the sqrt fills a bf16
            staging pair that a DMA scatters to partitions {0,64}."""
            sq = sb.tile([P, N], BF16, tag="liftsq")
            nc.vector.tensor_tensor(sq[:], dst[:], dst[:], MULT)
            nrm_t = psP.tile([P, 16, P], F32, tag="vps")
            nrm = nrm_t[:].rearrange("p (c u) m -> p c (u m)", u=4)
            for qc in range(4):
                nc.tensor.matmul(
                    nrm[0:2, qc, :],
                    lhsT=lift_mask[:, 0:2],
                    rhs=sq[:, qc * 512 : (qc + 1) * 512],
                    start=True,
                    stop=True,
                )
            ttmp = sb.tile([2, N], BF16, tag="ttmp")
            nc.scalar.activation(
                ttmp[:],
                nrm_t[0:2, :, :].rearrange("h c m -> h (c m)"),
                SQRT,
                bias=sc2bias[0:2, :],
                scale=1.0,
            )
            nc.scalar.dma_start(
                dst[:].rearrange("(h d) n -> h d n", h=2)[:, 0, :],
                ttmp[:],
            )

        # K and V projections first (independent psum tags keep the PE fed),
        # then the K lift, Q projection + lift. V's transpose to natural
        # layout is deferred into the phase-B psT pool.
        project_T(xs, "wk", b_sb["bk"], k_sb, "kqps")
        vps = psP.tile([P, 16, P], F32, tag="vps")
        # 4 mc slots share each 2KB psum zero-region: only the first matmul
        # of a bank may set start (it marks the whole region pending-zero).
        for pr in range(2):
            for mc in range(16):
                nc.tensor.matmul(
                    vps[:, mc, :],
                    lhsT=xs[:, 2 * pr : 2 * pr + 2, mc * P : (mc + 1) * P],
                    rhs=w_sb["wv"][:, 2 * pr : 2 * pr + 2, :],
                    start=(pr == 0 and mc % 4 == 0),
                    stop=(pr == 1 and mc % 4 == 3),
                    perf_mode=DR,
                    skip_group_check=True,
                )
        nc.vector.tensor_copy(
            out=v_sb[:].rearrange("p a b m -> p (a b m)"),
            in_=vps[:].rearrange("p a m -> p (a m)"),
        )
        lift_T(k_sb)
        project_T(xq, "wq", b_sb["bq"], q_sb, "kqps")
        lift_T(q_sb)
        ctxA.close()

        psS = ctx.enter_context(tc.tile_pool(name="psS", bufs=2, space="PSUM"))
        psPV = ctx.enter_context(tc.tile_pool(name="psPV", bufs=1, space="PSUM"))
        psT = ctx.enter_context(tc.tile_pool(name="psT", bufs=2, space="PSUM"))

        vsq = sb.tile([P, 8, 2, P], BF16, tag="vsq")
        nc.vector.tensor_tensor(vsq[:], v_sb[:], v_sb[:], MULT)
        vn = sb.tile([P, 8, 2, 2, 1], F32, tag="vn")
        nc.vector.tensor_reduce(
            vn[:, :, :, :, 0],
            vsq[:].rearrange("p a b (h d) -> p a b h d", h=2),
            axis=mybir.AxisListType.X,
            op=ADD,
        )
        nc.scalar.activation(
            v_sb[:].rearrange("p a b (h d) -> p a b h d", h=2)[:, :, :, :, 0:1],
            vn[:],
            SQRT,
            bias=sc2bias[:],
            scale=1.0,
        )
        # ebias = 0*v_sb[...] + act_bias: pins every exp behind the V lift so
        # the ACT queue runs all Sqrt calls before switching to the Exp table.
        nc.vector.tensor_scalar(
            ebias[:], v_sb[:, 0, 0, 0:1], 0.0, 2.0 / scale_val + bias_val,
            MULT, ADD,
        )

        # ---- Phase B/C: attention with deferred PV + per-half ReduceScatter ----
        pP = ctx.enter_context(tc.tile_pool(name="pP", bufs=32))
        dram = ctx.enter_context(tc.tile_pool(name="dram", bufs=1, space="DRAM"))

        act_scale = -2.0 / (scale_val * SC2)

        # dummy exp: prefetch the exp table right after the sqrt epoch
        junk = sb.tile([P, 1], F32, tag="junk")
        nc.scalar.activation(junk[:], ebias[:], EXP, scale=1.0)

        cc_outs = []
        pending_rs = []

        def emit_rs(half, hs_half):
            cc_in = dram.tile([4, 256, 64], F32, name=f"cc_in{half}")
            cc_out = dram.tile([256, 64], F32, name=f"cc_out{half}")
            nc.sync.dma_start(
                cc_in[:].rearrange("g (a p) d -> p g a d", p=P),
                hs_half[:].rearrange("p (g a) d -> p g a d", g=4),
            )
            nc.gpsimd.collective_compute(
                "ReduceScatter",
                ADD,
                replica_groups=REPLICA_GROUPS,
                ins=[cc_in[:].opt()],
                outs=[cc_out[:].opt()],
            )
            cc_outs.append(cc_out)

        for half in range(2):
            q0 = half * 1024
            # -- scores + exp: all 16 key tiles x 2 heads x 2 query chunks --
            p_ts = {}
            for pr in range(8):
                for h in range(2):
                    p_t = pP.tile([P, 2, 2, 512], FP8, tag="p", name=f"pt{half}_{pr}_{h}")
                    p_ts[(pr, h)] = p_t
                    for qcl in range(2):
                        s_ps = psS.tile([P, 2, 512], F32, tag="s")
                        for mci in range(2):
                            mc = 2 * pr + mci
                            nc.tensor.matmul(
                                s_ps[:, mci, :],
                                lhsT=k_sb[
                                    h * 64 : (h + 1) * 64, mc * P : (mc + 1) * P
                                ],
                                rhs=q_sb[
                                    h * 64 : (h + 1) * 64,
                                    q0 + qcl * 512 : q0 + (qcl + 1) * 512,
                                ],
                                start=True,
                                stop=True,
                            )
                        nc.scalar.activation(
                            p_t[:, qcl, :, :], s_ps[:], EXP,
                            scale=act_scale, bias=ebias[:],
                        )

            # previous half's ReduceScatter: emitted only now so none of this
            # half's PV bookkeeping lands behind the blocking collective on
            # the gpsimd queue
            if pending_rs:
                emit_rs(*pending_rs.pop())

            # -- PV + per-head centroid + head-sum, one 512-query chunk at a --
            # -- time (pv psum tile is drained before the next chunk reuses) --
            hs_half = sb.tile([P, 8, 64], F32, tag=f"hs_half{half}")
            for qcl in range(2):
                pv = psPV.tile([64, 2, 512], F32, tag="pv")
                for pr in range(8):
                    for h in range(2):
                        nc.tensor.matmul(
                            pv[:, h, :],
                            lhsT=v_sb[:, pr, :, h * 64 : (h + 1) * 64],
                            rhs=p_ts[(pr, h)][:, qcl, :, :],
                            start=(pr == 0),
                            stop=(pr == 7),
                            perf_mode=DR,
                            skip_group_check=True,
                        )
                o_bf = sb.tile([64, 2, 512], BF16, tag="o_bf")
                nc.vector.tensor_copy(
                    out=o_bf[:].rearrange("p a n -> p (a n)"),
                    in_=pv[:].rearrange("p a n -> p (a n)"),
                )
                ps_t = psT.tile([P, 2, 4, 64], BF16, tag="ot")
                for h in range(2):
                    for c in range(4):
                        nc.tensor.transpose(
                            ps_t[:, h, c, :],
                            o_bf[:, h, c * P : (c + 1) * P],
                            ident[0:64, 0:64],
                        )
                o_nat = sb.tile([P, 4, P], BF16, tag="o_nat")
                nc.vector.tensor_copy(
                    out=o_nat[:].rearrange("p c (h d) -> p h c d", h=2),
                    in_=ps_t[:],
                )
                sq = sb.tile([P, 4, P], BF16, tag="sq_nat")
                nc.vector.tensor_tensor(sq[:], o_nat[:], o_nat[:], MULT)
                p2 = sb.tile([P, 4, 2, 1], F32, tag="p2")
                nc.vector.tensor_reduce(
                    p2[:, :, :, 0],
                    sq[:].rearrange("p a (h d) -> p a h d", h=2),
                    axis=mybir.AxisListType.X,
                    op=ADD,
                )
                tbar = sb.tile([P, 4, 2, 1], F32, tag="tbar")
                nc.vector.tensor_copy(
                    out=tbar[:].rearrange("p a h one -> p (a h one)"),
                    in_=o_nat[:].rearrange("p a (h d) -> p a h d", h=2)[
                        :, :, :, 0:1
                    ].rearrange("p a h one -> p (a h one)"),
                )
                # delta = colsum - t^2 with the *same* bf16-rounded t^2 from
                # sq, so the dominant terms cancel exactly.
                fl = lambda ap: ap.rearrange("p a h one -> p (a h) one")
                delta = sb.tile([P, 4, 2, 1], F32, tag="delta")
                nc.vector.affine_then_add(
                    out=fl(delta[:]),
                    in0=sq[:].rearrange("p a (h d) -> p (a h) d", h=2)[:, :, 0:1],
                    in1=fl(p2[:]),
                    scale=-1.0,
                    bias=0.0,
                )
                rt = sb.tile([P, 4, 2, 1], F32, tag="rt")
                nc.vector.reciprocal_approx_fast(fl(rt[:]), fl(tbar[:]))
                z = sb.tile([P, 4, 2, 1], F32, tag="z")
                nc.vector.tensor_tensor(z[:], delta[:], rt[:], MULT)
                den = sb.tile([P, 4, 2, 1], F32, tag="den")
                nc.vector.affine_then_add(
                    out=fl(den[:]), in0=fl(z[:]), in1=fl(tbar[:]),
                    scale=-0.5, bias=0.0,
                )
                rec = sb.tile([P, 4, 2, 1], F32, tag="rec")
                nc.vector.reciprocal_approx_fast(fl(rec[:]), fl(den[:]))
                o4 = o_nat[:].rearrange("p a (h d) -> p a h d", h=2)
                m0 = sb.tile([P, 4, 64], F32, tag="m0")
                nc.vector.tensor_tensor(
                    m0[:], o4[:, :, 0, :],
                    rec[:, :, 0, :].to_broadcast((P, 4, 64)), MULT,
                )
                m1 = sb.tile([P, 4, 64], F32, tag="m1")
                nc.vector.tensor_tensor(
                    m1[:], o4[:, :, 1, :],
                    rec[:, :, 1, :].to_broadcast((P, 4, 64)), MULT,
                )
                nc.vector.tensor_tensor(
                    hs_half[:, qcl * 4 : (qcl + 1) * 4, :], m0[:], m1[:], ADD
                )

            pending_rs.append((half, hs_half))
        emit_rs(*pending_rs.pop())

        # ---- final centroid on the two local 256-query slices (natural) ----
        for half in range(2):
            fin = sb.tile([P, 2, 64], F32, tag="fin")
            nc.scalar.dma_start(
                fin[:], cc_outs[half][:].rearrange("(a p) d -> p a d", p=P)
            )
            fsq = sb.tile([P, 2, 64], BF16, tag="fsq")
            nc.vector.tensor_tensor(fsq[:], fin[:], fin[:], MULT)
            fp2 = sb.tile([P, 2, 1], F32, tag="fp2")
            nc.vector.tensor_reduce(
                fp2[:, :, 0], fsq[:], axis=mybir.AxisListType.X, op=ADD
            )
            fdel = sb.tile([P, 2, 1], F32, tag="fdel")
            nc.vector.affine_then_add(
                out=fdel[:], in0=fsq[:, :, 0:1], in1=fp2[:], scale=-1.0, bias=0.0
            )
            frt = sb.tile([P, 2, 1], F32, tag="frt")
            nc.vector.reciprocal_approx_fast(frt[:], fin[:, :, 0:1])
            fz = sb.tile([P, 2, 1], F32, tag="fz")
            nc.vector.tensor_tensor(fz[:], fdel[:], frt[:], MULT)
            fden = sb.tile([P, 2, 1], F32, tag="fden")
            nc.vector.affine_then_add(
                out=fden[:], in0=fz[:], in1=fin[:, :, 0:1], scale=-0.5, bias=0.0
            )
            frec = sb.tile([P, 2, 1], F32, tag="frec")
            nc.vector.reciprocal_approx_fast(frec[:], fden[:])
            out_sb = sb.tile([P, 2, 64], F32, tag="out_sb")
            nc.vector.tensor_tensor(
                out_sb[:], fin[:], frec[:].to_broadcast((P, 2, 64)), MULT
            )
            nc.scalar.dma_start(
                io["out"].ap()[half, :, :].rearrange("(a p) d -> p a d", p=P),
                out_sb[:],
            )


def _build(scale_val, bias_val):
    nc = bacc.Bacc(num_devices=N_CORES)
    io = {}
    io["xq"] = nc.declare_dram_parameter("xq", [E, N], FP8, isOutput=False)
    io["xs"] = nc.declare_dram_parameter("xs", [E, N], FP8, isOutput=False)
    for nm in ("wq", "wk", "wv"):
        io[nm] = nc.declare_dram_parameter(nm, [E, P], FP8, isOutput=False)
    for nm in ("bq", "bk"):
        io[nm] = nc.declare_dram_parameter(nm, [P], F32, isOutput=False)
    io["lift_mask"] = nc.declare_dram_parameter("lift_mask", [P, P], BF16, isOutput=False)
    io["ident"] = nc.declare_dram_parameter("ident", [P, P], BF16, isOutput=False)
    io["out"] = nc.declare_dram_parameter("out", [2, 256, 64], F32, isOutput=True)

    with tile.TileContext(nc) as tc:
        _emit(tc, nc, io, scale_val, bias_val)
    nc.compile()
    return nc


_BUILD_CACHE = {}


def _get_nc(scale_val, bias_val):
    key = (float(scale_val), float(bias_val))
    if key not in _BUILD_CACHE:
        _BUILD_CACHE[key] = _build(*key)
    return _BUILD_CACHE[key]


def _pad_wT(w_heads):
    """w_heads: [126, 512] spatial weights for 2 heads -> [512, 128] transposed
    with zero columns at 0 and 64 (time slots), scaled by W_SCALE."""
    out = np.zeros((E, P), dtype=np.float32)
    out[:, 1:64] = W_SCALE * w_heads[0:DM1, :].T
    out[:, 65:128] = W_SCALE * w_heads[DM1 : 2 * DM1, :].T
    return np.ascontiguousarray(out)


def _pad_b(b_heads):
    out = np.zeros((P,), dtype=np.float32)
    out[1:64] = W_SCALE * b_heads[0:DM1]
    out[65:128] = W_SCALE * b_heads[DM1 : 2 * DM1]
    return out


def make_in_maps(
    query_input, source_input, Wq_w, Wq_b, Wk_w, Wk_b, Wv_w, Wv_b, scale, bias
):
    import ml_dtypes

    F8 = ml_dtypes.float8_e4m3fn
    BF = ml_dtypes.bfloat16

    lift_mask = np.zeros((P, P), dtype=np.float32)
    lift_mask[1:64, 0] = 1.0
    lift_mask[65:128, 1] = 1.0
    ident = np.eye(P, dtype=np.float32)

    in_maps = []
    for c in range(N_CORES):
        b = c // 4
        h0 = 2 * (c % 4)
        sl = slice(h0 * DM1, (h0 + 2) * DM1)
        m = {
            "xq": np.ascontiguousarray(query_input[b].T).astype(F8),
            "xs": np.ascontiguousarray(source_input[b].T).astype(F8),
            "wq": _pad_wT(Wq_w[sl]).astype(F8),
            "wk": _pad_wT(-Wk_w[sl]).astype(F8),  # Lorentz sign folded into K
            "wv": _pad_wT(Wv_w[sl]).astype(F8),
            "bq": _pad_b(Wq_b[sl]),
            "bk": _pad_b(-Wk_b[sl]),
            "lift_mask": lift_mask.astype(BF),
            "ident": ident.astype(BF),
        }
        in_maps.append(m)
    return in_maps


def kernel(
    query_input,
    source_input,
    Wq_w,
    Wq_b,
    Wk_w,
    Wk_b,
    Wv_w,
    Wv_b,
    scale,
    bias,
    _trace=False,
):
    assert not np.any(np.asarray(Wv_b)), "nonzero V bias not supported"
    scale_val = float(np.asarray(scale).reshape(-1)[0])
    bias_val = float(np.asarray(bias).reshape(-1)[0]) if np.asarray(bias).size else 0.0

    nc = _get_nc(scale_val, bias_val)
    in_maps = make_in_maps(
        query_input, source_input, Wq_w, Wq_b, Wk_w, Wk_b, Wv_w, Wv_b, scale, bias
    )

    from concourse.bass_utils import run_bass_kernel_spmd

    res = run_bass_kernel_spmd(
        nc, in_maps, core_ids=list(range(N_CORES)), trace=_trace
    )

    out = np.zeros((B, N, D), dtype=np.float32)
    for c in range(N_CORES):
        b = c // 4
        g = c % 4
        r = res.results[c]["out"]  # [2, 256, 64]
        for half in range(2):
            q0 = half * 1024 + g * 256
            out[b, q0 : q0 + 256, :] = r[half]
    if _trace:
        kernel.last_exec_time_ns = res.exec_time_ns
        kernel.last_results = res
    return out


# revision 34
# speedup vs baseline: 1.2325x; 1.0206x over previous
"""Trainium2 Bass kernel for LorentzMultiheadAttention (B=2, N=2048, H=8, D=64, E=512).

Sharding: 8 cores = 2 batches x 4 head-pairs. Core c handles batch b=c//4 and
heads {2*(c%4), 2*(c%4)+1}. Queries run through the attention pipeline in 4
quarters of 512; each quarter's per-head centroid + head-sum is
ReduceScattered over the 4-core batch group while later quarters compute,
hiding the collectives.

Key design points:
- PV and the projections use fp8e4 DoubleRow matmuls (2 contraction rows per
  cycle): PV pairs adjacent key tiles, projections pair E=512/128 subtiles.
  Scores (contract=64) gain nothing from DoubleRow, so they stay bf16.
- Weights are scaled by 32 so fp8e4 stays in normal range. The scale cancels:
  the Lorentz centroid is scale-invariant, the lift becomes
  t = sqrt(32^2 + ||x'||^2), and the score scale folds into the exp activation
  scale. The Lorentz sign is folded into negated K weights (host-side).
- Softmax exp writes fp8 directly (ACT output cast) in key-tile pairs that
  feed PV's DoubleRow layout; softmax normalization and the mean-over-heads
  divide are skipped (centroid scale-invariance).
- The epilogue transposes each quarter's PV output to natural [q, d] layout
  on the PE, computes 1/sqrt(|<ave,ave>|) ~= 1/(t - delta/(2t)) with DVE
  reciprocal_approx_fast (output is heavily time-dominated; delta uses the
  same bf16-rounded t^2 so the cancellation is exact), and reduces heads with
  a free-dim broadcast multiply. No ACT usage after the softmax exps: exactly
  2 activation-table loads (sqrt epoch, then exp).
- A burst of dummy matmuls at kernel start warms the PE HAM clock gate while
  inputs DMA in; a dummy exp right after the lift sqrts prefetches the exp
  table before the first score tile lands.
"""

import os
import sys

for _p in ("/opt/trn_rl_repo", "/root/.axon_site/_ro/trn_rl_repo"):
    if os.path.isdir(_p) and _p not in sys.path:
        sys.path.insert(0, _p)

import numpy as np

import concourse.bacc as bacc
import concourse.bass as bass
import concourse.mybir as mybir
import concourse.tile as tile

B = 2
N = 2048
H = 8
D = 64
E = 512
DM1 = D - 1  # 63
P = 128
N_CORES = 8
NQTR = 4
QTR = N // NQTR  # 512 queries per quarter
QB = 128  # query block per core from one quarter's ReduceScatter

W_SCALE = 32.0
SC2 = W_SCALE * W_SCALE  # 1024

F32 = mybir.dt.float32
BF16 = mybir.dt.bfloat16
FP8 = mybir.dt.float8e4
EXP = mybir.ActivationFunctionType.Exp
SQRT = mybir.ActivationFunctionType.Sqrt
ADD = mybir.AluOpType.add
MULT = mybir.AluOpType.mult
DR = mybir.MatmulPerfMode.DoubleRow

REPLICA_GROUPS = [[0, 1, 2, 3], [4, 5, 6, 7]]


def _emit(tc, nc, io, scale_val, bias_val):
    from contextlib import ExitStack

    ctx = ExitStack()
    with ctx:
        consts = ctx.enter_context(tc.tile_pool(name="consts", bufs=1))
        sb = ctx.enter_context(tc.tile_pool(name="sb", bufs=1))

        # ---- constants / weights to SBUF ----
        w_sb = {}
        for nm in ("wq", "wk", "wv"):
            w = consts.tile([P, 4, P], FP8, name=f"{nm}_sb")
            nc.sync.dma_start(w[:], io[nm].ap().rearrange("(c p) m -> p c m", p=P))
            w_sb[nm] = w
        b_sb = {}
        for nm in ("bq", "bk"):
            bt = consts.tile([P, 1], F32, name=f"{nm}_sb")
            nc.sync.dma_start(bt[:], io[nm].ap().rearrange("(p one) -> p one", one=1))
            b_sb[nm] = bt
        lift_mask = consts.tile([P, P], BF16)
        nc.sync.dma_start(lift_mask[:], io["lift_mask"].ap())
        ident = consts.tile([P, P], BF16)
        nc.sync.dma_start(ident[:], io["ident"].ap())
        sc2bias = consts.tile([P, 1], F32)
        nc.gpsimd.memset(sc2bias[:], SC2)
        ebias = consts.tile([P, 1], F32)

        # ---- warm-up matmuls: keep the PE HAM gate busy during input DMA ----
        ctxW = ExitStack()
        psW = ctxW.enter_context(tc.tile_pool(name="psW", bufs=1, space="PSUM"))
        warm = psW.tile([P, P], F32, tag="warm")
        for _ in range(20):
            nc.tensor.matmul(
                warm[:], lhsT=lift_mask[:], rhs=lift_mask[:], start=True, stop=True
            )
        ctxW.close()

        # ---- inputs (fp8, chunked DMA so projections can start early) ----
        xs = sb.tile([P, 4, N], FP8)
        xq = sb.tile([P, 4, N], FP8)
        for ec in range(4):
            nc.sync.dma_start(
                xs[:, ec : ec + 1, :],
                io["xs"].ap().rearrange("(c p) n -> p c n", p=P)[:, ec : ec + 1, :],
            )
        for ec in range(4):
            nc.sync.dma_start(
                xq[:, ec : ec + 1, :],
                io["xq"].ap().rearrange("(c p) n -> p c n", p=P)[:, ec : ec + 1, :],
            )

        # q/k: [128 (h*64+d), 2048] bf16, rows 0/64 become the lift time rows
        q_sb = sb.tile([P, N], BF16)
        k_sb = sb.tile([P, N], BF16)
        # v: [128 keys, 8 pairs, 2 (key tile in pair), 128 (h*64+d)] fp8
        v_sb = sb.tile([P, 8, 2, P], FP8)

        ctxA = ExitStack()
        psP = ctxA.enter_context(tc.tile_pool(name="psP", bufs=1, space="PSUM"))

        def project_T(x_sb, wname, bias, dst, tag):
            """Transposed projection: dst = W^T x + b (both heads)."""
            ps = psP.tile([P, 4, 512], F32, tag=tag)
            for pr in range(2):
                for qc in range(4):
                    nc.tensor.matmul(
                        ps[:, qc, :],
                        lhsT=w_sb[wname][:, 2 * pr : 2 * pr + 2, :],
                        rhs=x_sb[:, 2 * pr : 2 * pr + 2, qc * 512 : (qc + 1) * 512],
                        start=(pr == 0),
                        stop=(pr == 1),
                        perf_mode=DR,
                    )
            nc.vector.tensor_tensor(
                dst[:],
                ps[:].rearrange("p c n -> p (c n)"),
                bias[:].to_broadcast((P, N)),
                ADD,
            )

        def lift_T(dst):
            """Write t = sqrt(1024 + ||x_s||^2) into rows 0/64 of dst.

            The norm matmul lands both heads' norms on partitions {0,1}
            (engines can't use strided partition APs); the sqrt fills a bf16
            staging pair that a DMA scatters to partitions {0,64}."""
            sq = sb.tile([P, N], BF16, tag="liftsq")
            nc.vector.tensor_tensor(sq[:], dst[:], dst[:], MULT)
            nrm_t = psP.tile([P, 16, P], F32, tag="vps")
            nrm = nrm_t[:].rearrange("p (c u) m -> p c (u m)", u=4)
            for qc in range(4):
                nc.tensor.matmul(
                    nrm[0:2, qc, :],
                    lhsT=lift_mask[:, 0:2],
                    rhs=sq[:, qc * 512 : (qc + 1) * 512],
                    start=True,
                    stop=True,
                )
            ttmp = sb.tile([2, N], BF16, tag="ttmp")
            nc.scalar.activation(
                ttmp[:],
                nrm_t[0:2, :, :].rearrange("h c m -> h (c m)"),
                SQRT,
                bias=sc2bias[0:2, :],
                scale=1.0,
            )
            nc.scalar.dma_start(
                dst[:].rearrange("(h d) n -> h d n", h=2)[:, 0, :],
                ttmp[:],
            )

        # K and V projections first (independent psum tags keep the PE fed),
        # then the K lift, Q projection + lift. V's transpose to natural
        # layout is deferred into the phase-B psT pool.
        project_T(xs, "wk", b_sb["bk"], k_sb, "kqps")
        vps = psP.tile([P, 16, P], F32, tag="vps")
        # 4 mc slots share each 2KB psum zero-region: only the first matmul
        # of a bank may set start (it marks the whole region pending-zero).
        for pr in range(2):
            for mc in range(16):
                nc.tensor.matmul(
                    vps[:, mc, :],
                    lhsT=xs[:, 2 * pr : 2 * pr + 2, mc * P : (mc + 1) * P],
                    rhs=w_sb["wv"][:, 2 * pr : 2 * pr + 2, :],
                    start=(pr == 0 and mc % 4 == 0),
                    stop=(pr == 1 and mc % 4 == 3),
                    perf_mode=DR,
                    skip_group_check=True,
                )
        nc.vector.tensor_copy(
            out=v_sb[:].rearrange("p a b m -> p (a b m)"),
            in_=vps[:].rearrange("p a m -> p (a m)"),
        )
        lift_T(k_sb)
        project_T(xq, "wq", b_sb["bq"], q_sb, "kqps")
        lift_T(q_sb)
        ctxA.close()

        psS = ctx.enter_context(tc.tile_pool(name="psS", bufs=2, space="PSUM"))
        psPV = ctx.enter_context(tc.tile_pool(name="psPV", bufs=1, space="PSUM"))
        psT = ctx.enter_context(tc.tile_pool(name="psT", bufs=2, space="PSUM"))

        vsq = sb.tile([P, 8, 2, P], BF16, tag="vsq")
        nc.vector.tensor_tensor(vsq[:], v_sb[:], v_sb[:], MULT)
        vn = sb.tile([P, 8, 2, 2, 1], F32, tag="vn")
        nc.vector.tensor_reduce(
            vn[:, :, :, :, 0],
            vsq[:].rearrange("p a b (h d) -> p a b h d", h=2),
            axis=mybir.AxisListType.X,
            op=ADD,
        )
        nc.scalar.activation(
            v_sb[:].rearrange("p a b (h d) -> p a b h d", h=2)[:, :, :, :, 0:1],
            vn[:],
            SQRT,
            bias=sc2bias[:],
            scale=1.0,
        )
        # ebias = 0*v_sb[...] + act_bias: pins every exp behind the V lift so
        # the ACT queue runs all Sqrt calls before switching to the Exp table.
        nc.vector.tensor_scalar(
            ebias[:], v_sb[:, 0, 0, 0:1], 0.0, 2.0 / scale_val + bias_val,
            MULT, ADD,
        )

        # ---- Phase B/C: attention with deferred PV + per-half ReduceScatter ----
        pP = ctx.enter_context(tc.tile_pool(name="pP", bufs=32))
        dram = ctx.enter_context(tc.tile_pool(name="dram", bufs=1, space="DRAM"))

        act_scale = -2.0 / (scale_val * SC2)

        # dummy exp: prefetch the exp table right after the sqrt epoch
        junk = sb.tile([P, 1], F32, tag="junk")
        nc.scalar.activation(junk[:], ebias[:], EXP, scale=1.0)

        cc_outs = []
        pending_rs = []

        def emit_rs(half, hs_half):
            cc_in = dram.tile([4, 256, 64], F32, name=f"cc_in{half}")
            cc_out = dram.tile([256, 64], F32, name=f"cc_out{half}")
            nc.sync.dma_start(
                cc_in[:].rearrange("g (a p) d -> p g a d", p=P),
                hs_half[:].rearrange("p (g a) d -> p g a d", g=4),
            )
            nc.gpsimd.collective_compute(
                "ReduceScatter",
                ADD,
                replica_groups=REPLICA_GROUPS,
                ins=[cc_in[:].opt()],
                outs=[cc_out[:].opt()],
            )
            cc_outs.append(cc_out)

        for half in range(2):
            q0 = half * 1024
            # -- scores + exp: all 16 key tiles x 2 heads x 2 query chunks --
            p_ts = {}
            for pr in range(8):
                for h in range(2):
                    p_t = pP.tile([P, 2, 2, 512], FP8, tag="p", name=f"pt{half}_{pr}_{h}")
                    p_ts[(pr, h)] = p_t
                    for qcl in range(2):
                        s_ps = psS.tile([P, 2, 512], F32, tag="s")
                        for mci in range(2):
                            mc = 2 * pr + mci
                            nc.tensor.matmul(
                                s_ps[:, mci, :],
                                lhsT=k_sb[
                                    h * 64 : (h + 1) * 64, mc * P : (mc + 1) * P
                                ],
                                rhs=q_sb[
                                    h * 64 : (h + 1) * 64,
                                    q0 + qcl * 512 : q0 + (qcl + 1) * 512,
                                ],
                                start=True,
                                stop=True,
                            )
                        nc.scalar.activation(
                            p_t[:, qcl, :, :], s_ps[:], EXP,
                            scale=act_scale, bias=ebias[:],
                        )

            # previous half's ReduceScatter: emitted only now so none of this
            # half's PV bookkeeping lands behind the blocking collective on
            # the gpsimd queue
            if pending_rs:
                emit_rs(*pending_rs.pop())

            # -- PV + per-head centroid + head-sum, one 512-query chunk at a --
            # -- time (pv psum tile is drained before the next chunk reuses) --
            hs_half = sb.tile([P, 8, 64], F32, tag=f"hs_half{half}")
            for qcl in range(2):
                pv = psPV.tile([64, 2, 512], F32, tag="pv")
                for pr in range(8):
                    for h in range(2):
                        nc.tensor.matmul(
                            pv[:, h, :],
                            lhsT=v_sb[:, pr, :, h * 64 : (h + 1) * 64],
                            rhs=p_ts[(pr, h)][:, qcl, :, :],
                            start=(pr == 0),
                            stop=(pr == 7),
                            perf_mode=DR,
                            skip_group_check=True,
                        )
                o_bf = sb.tile([64, 2, 512], BF16, tag="o_bf")
                nc.vector.tensor_copy(
                    out=o_bf[:].rearrange("p a n -> p (a n)"),
                    in_=pv[:].rearrange("p a n -> p (a n)"),
                )
                ps_t = psT.tile([P, 2, 4, 64], BF16, tag="ot")
                for h in range(2):
                    for c in range(4):
                        nc.tensor.transpose(
                            ps_t[:, h, c, :],
                            o_bf[:, h, c * P : (c + 1) * P],
                            ident[0:64, 0:64],
                        )
                o_nat = sb.tile([P, 4, P], BF16, tag="o_nat")
                nc.vector.tensor_copy(
                    out=o_nat[:].rearrange("p c (h d) -> p h c d", h=2),
                    in_=ps_t[:],
                )
                sq = sb.tile([P, 4, P], BF16, tag="sq_nat")
                nc.vector.tensor_tensor(sq[:], o_nat[:], o_nat[:], MULT)
                p2 = sb.tile([P, 4, 2, 1], F32, tag="p2")
                nc.vector.tensor_reduce(
                    p2[:, :, :, 0],
                    sq[:].rearrange("p a (h d) -> p a h d", h=2),
                    axis=mybir.AxisListType.X,
                    op=ADD,
                )
                tbar = sb.tile([P, 4, 2, 1], F32, tag="tbar")
                nc.vector.tensor_copy(
                    out=tbar[:].rearrange("p a h one -> p (a h one)"),
                    in_=o_nat[:].rearrange("p a (h d) -> p a h d", h=2)[
                        :, :, :, 0:1
                    ].rearrange("p a h one -> p (a h one)"),
                )
                # delta = colsum - t^2 with the *same* bf16-rounded t^2 from
                # sq, so the dominant terms cancel exactly.
                fl = lambda ap: ap.rearrange("p a h one -> p (a h) one")
                delta = sb.tile([P, 4, 2, 1], F32, tag="delta")
                nc.vector.affine_then_add(
                    out=fl(delta[:]),
                    in0=sq[:].rearrange("p a (h d) -> p (a h) d", h=2)[:, :, 0:1],
                    in1=fl(p2[:]),
                    scale=-1.0,
                    bias=0.0,
                )
                rt = sb.tile([P, 4, 2, 1], F32, tag="rt")
                nc.vector.reciprocal_approx_fast(fl(rt[:]), fl(tbar[:]))
                z = sb.tile([P, 4, 2, 1], F32, tag="z")
                nc.vector.tensor_tensor(z[:], delta[:], rt[:], MULT)
                den = sb.tile([P, 4, 2, 1], F32, tag="den")
                nc.vector.affine_then_add(
                    out=fl(den[:]), in0=fl(z[:]), in1=fl(tbar[:]),
                    scale=-0.5, bias=0.0,
                )
                rec = sb.tile([P, 4, 2, 1], F32, tag="rec")
                nc.vector.reciprocal_approx_fast(fl(rec[:]), fl(den[:]))
                o4 = o_nat[:].rearrange("p a (h d) -> p a h d", h=2)
                m0 = sb.tile([P, 4, 64], F32, tag="m0")
                nc.vector.tensor_tensor(
                    m0[:], o4[:, :, 0, :],
                    rec[:, :, 0, :].to_broadcast((P, 4, 64)), MULT,
                )
                m1 = sb.tile([P, 4, 64], F32, tag="m1")
                nc.vector.tensor_tensor(
                    m1[:], o4[:, :, 1, :],
                    rec[:, :, 1, :].to_broadcast((P, 4, 64)), MULT,
                )
                nc.vector.tensor_tensor(
                    hs_half[:, qcl * 4 : (qcl + 1) * 4, :], m0[:], m1[:], ADD
                )

            pending_rs.append((half, hs_half))
        emit_rs(*pending_rs.pop())

        # ---- final centroid on the two local 256-query slices (natural) ----
        for half in range(2):
            fin = sb.tile([P, 2, 64], F32, tag="fin")
            nc.scalar.dma_start(
                fin[:], cc_outs[half][:].rearrange("(a p) d -> p a d", p=P)
            )
            fsq = sb.tile([P, 2, 64], BF16, tag="fsq")
            nc.vector.tensor_tensor(fsq[:], fin[:], fin[:], MULT)
            fp2 = sb.tile([P, 2, 1], F32, tag="fp2")
            nc.vector.tensor_reduce(
                fp2[:, :, 0], fsq[:], axis=mybir.AxisListType.X, op=ADD
            )
            fdel = sb.tile([P, 2, 1], F32, tag="fdel")
            nc.vector.affine_then_add(
                out=fdel[:], in0=fsq[:, :, 0:1], in1=fp2[:], scale=-1.0, bias=0.0
            )
            frt = sb.tile([P, 2, 1], F32, tag="frt")
            nc.vector.reciprocal_approx_fast(frt[:], fin[:, :, 0:1])
            fz = sb.tile([P, 2, 1], F32, tag="fz")
            nc.vector.tensor_tensor(fz[:], fdel[:], frt[:], MULT)
            fden = sb.tile([P, 2, 1], F32, tag="fden")
            nc.vector.affine_then_add(
                out=fden[:], in0=fz[:], in1=fin[:, :, 0:1], scale=-0.5, bias=0.0
            )
            frec = sb.tile([P, 2, 1], F32, tag="frec")
            nc.vector.reciprocal_approx_fast(frec[:], fden[:])
            out_sb = sb.tile([P, 2, 64], F32, tag="out_sb")
            nc.vector.tensor_tensor(
                out_sb[:], fin[:], frec[:].to_broadcast((P, 2, 64)), MULT
            )
            nc.scalar.dma_start(
                io["out"].ap()[half, :, :].rearrange("(a p) d -> p a d", p=P),
                out_sb[:],
            )


def _build(scale_val, bias_val):
    nc = bacc.Bacc(num_devices=N_CORES)
    io = {}
    io["xq"] = nc.declare_dram_parameter("xq", [E, N], FP8, isOutput=False)
    io["xs"] = nc.declare_dram_parameter("xs", [E, N], FP8, isOutput=False)
    for nm in ("wq", "wk", "wv"):
        io[nm] = nc.declare_dram_parameter(nm, [E, P], FP8, isOutput=False)
    for nm in ("bq", "bk"):
        io[nm] = nc.declare_dram_parameter(nm, [P], F32, isOutput=False)
    io["lift_mask"] = nc.declare_dram_parameter("lift_mask", [P, P], BF16, isOutput=False)
    io["ident"] = nc.declare_dram_parameter("ident", [P, P], BF16, isOutput=False)
    io["out"] = nc.declare_dram_parameter("out", [2, 256, 64], F32, isOutput=True)

    with tile.TileContext(nc) as tc:
        _emit(tc, nc, io, scale_val, bias_val)
    nc.compile()
    return nc


_BUILD_CACHE = {}


def _get_nc(scale_val, bias_val):
    key = (float(scale_val), float(bias_val))
    if key not in _BUILD_CACHE:
        _BUILD_CACHE[key] = _build(*key)
    return _BUILD_CACHE[key]


def _pad_wT(w_heads):
    """w_heads: [126, 512] spatial weights for 2 heads -> [512, 128] transposed
    with zero columns at 0 and 64 (time slots), scaled by W_SCALE."""
    out = np.zeros((E, P), dtype=np.float32)
    out[:, 1:64] = W_SCALE * w_heads[0:DM1, :].T
    out[:, 65:128] = W_SCALE * w_heads[DM1 : 2 * DM1, :].T
    return np.ascontiguousarray(out)


def _pad_b(b_heads):
    out = np.zeros((P,), dtype=np.float32)
    out[1:64] = W_SCALE * b_heads[0:DM1]
    out[65:128] = W_SCALE * b_heads[DM1 : 2 * DM1]
    return out


def make_in_maps(
    query_input, source_input, Wq_w, Wq_b, Wk_w, Wk_b, Wv_w, Wv_b, scale, bias
):
    import ml_dtypes

    F8 = ml_dtypes.float8_e4m3fn
    BF = ml_dtypes.bfloat16

    lift_mask = np.zeros((P, P), dtype=np.float32)
    lift_mask[1:64, 0] = 1.0
    lift_mask[65:128, 1] = 1.0
    ident = np.eye(P, dtype=np.float32)

    in_maps = []
    for c in range(N_CORES):
        b = c // 4
        h0 = 2 * (c % 4)
        sl = slice(h0 * DM1, (h0 + 2) * DM1)
        m = {
            "xq": np.ascontiguousarray(query_input[b].T).astype(F8),
            "xs": np.ascontiguousarray(source_input[b].T).astype(F8),
            "wq": _pad_wT(Wq_w[sl]).astype(F8),
            "wk": _pad_wT(-Wk_w[sl]).astype(F8),  # Lorentz sign folded into K
            "wv": _pad_wT(Wv_w[sl]).astype(F8),
            "bq": _pad_b(Wq_b[sl]),
            "bk": _pad_b(-Wk_b[sl]),
            "lift_mask": lift_mask.astype(BF),
            "ident": ident.astype(BF),
        }
        in_maps.append(m)
    return in_maps


def kernel(
    query_input,
    source_input,
    Wq_w,
    Wq_b,
    Wk_w,
    Wk_b,
    Wv_w,
    Wv_b,
    scale,
    bias,
    _trace=False,
):
    assert not np.any(np.asarray(Wv_b)), "nonzero V bias not supported"
    scale_val = float(np.asarray(scale).reshape(-1)[0])
    bias_val = float(np.asarray(bias).reshape(-1)[0]) if np.asarray(bias).size else 0.0

    nc = _get_nc(scale_val, bias_val)
    in_maps = make_in_maps(
        query_input, source_input, Wq_w, Wq_b, Wk_w, Wk_b, Wv_w, Wv_b, scale, bias
    )

    from concourse.bass_utils import run_bass_kernel_spmd

    res = run_bass_kernel_spmd(
        nc, in_maps, core_ids=list(range(N_CORES)), trace=_trace
    )

    out = np.zeros((B, N, D), dtype=np.float32)
    for c in range(N_CORES):
        b = c // 4
        g = c % 4
        r = res.results[c]["out"]  # [2, 256, 64]
        for half in range(2):
            q0 = half * 1024 + g * 256
            out[b, q0 : q0 + 256, :] = r[half]
    if _trace:
        kernel.last_exec_time_ns = res.exec_time_ns
        kernel.last_results = res
    return out
